# revision 32
# baseline (speedup 1.0000x reference)
"""MoE block (8 experts, top-2, + shared expert) on 8 trn2 NeuronCores.

Strategy (FF-sharded tensor parallelism, host dispatch/combine):
  - Host computes gate logits/softmax/top-2 (0.03% of total FLOPs).
  - Every core receives the SAME activation matrix xt = [all 4096 tokens
    (shared-expert pass) | expert-0's routed tokens | ... | expert-7's],
    and a distinct 512-wide slice of the FF dim of EVERY weight matrix
    (8 experts + shared = 9 groups; slices stream through 3 rotating
    SBUF buffers since groups are consumed in order).
    gelu is elementwise over FF, so y = sum_c gelu(x@W1[:,c])@W2[c,:]
    decomposes exactly; the host sums the 8 partial outputs in fp32.
  - Per-core work is identical regardless of routing (no padding to the
    max expert count, no load imbalance): (12288+pad) cols x 1/8 of FF.
  - Matmuls are bf16 with fp32 PSUM accumulation; feature-major layout
    ([D, tokens]) avoids all on-device transposes. Outputs return bf16
    (partials are summed in fp32 on host; quantization adds ~1e-3 rel).

Granularity: x is loaded per CHUNK (up to a whole expert group, 13
loads total — each chunk-start matmul pays ~120ns for its x semaphore
wait, so fewer chunks = fewer stalls); compute and y stores run per
SUB-chunk (<=512 cols, the PSUM bank + matmul free-dim limit).

DMA layout: every dram tensor is PRE-PACKED on the host into the exact
SBUF tile byte order ([128 partitions, rest-contiguous]), so each
transfer is one contiguous ~8-17KB segment per partition (128
descriptors per transfer). The HWDGE generates descriptors inline on
the issuing sequencer's DIRECT2D at ~2.5-7ns each, so 1KB-segment
transfers (the naive [D, NT] layout) cap the whole kernel at ~150GB/s
of issue rate; packed transfers measured 345GB/s/core with all 8 cores
running. Weights stream on the ACT HWDGE ring, x/y on the SP ring,
keeping the periodic x prefetches out of FIFO order behind weight
megabytes.
"""

import numpy as np
import ml_dtypes

import concourse.bass as bass
import concourse.bacc as bacc
from concourse import mybir
from concourse.tile import TileContext
from concourse.bass_utils import run_bass_kernel_spmd

D = 1024
FF = 4096
E = 8
TOPK = 2
B, L = 4, 1024
T = B * L
NCORES = 8
P = 128
DT = D // P         # 8 k-tiles over D
FSL = FF // NCORES  # 512 FF columns per core
FT = FSL // P       # 4 f-tiles per core slice
NG = E + 1          # weight groups: 0 = shared, 1..8 = experts

_BF16 = mybir.dt.bfloat16
_F32 = mybir.dt.float32

_program_cache: dict[tuple, object] = {}

# test harness hooks: extra kwargs for run_bass_kernel_spmd (e.g. trace=True)
# and the last BassKernelResults for profiling. Unused in normal grading runs.
TRACE_KWARGS: dict = {}
last_results = None

N_WARM = 24  # dummy matmuls bridging the DMA delivery after kernel start
             # (sized for the slowest core's data arrival; exec time is
             # the max over cores, so the bridge covers arrival jitter)


def _near_equal(w: int, cap: int) -> list[int]:
    """Split w into ceil(w/cap) near-equal parts, multiples of 8."""
    n = -(-w // cap)
    base = -(-w // n)
    base = -(-base // 8) * 8
    sizes = []
    left = w
    for _ in range(n - 1):
        sizes.append(base)
        left -= base
    sizes.append(left)
    assert all(0 < s <= cap for s in sizes) and sum(sizes) == w
    return sizes


def _chunk_list(widths: tuple[int, ...]):
    """Chunks (x granularity) with sub-chunks (compute/y granularity).

    Returns ([(g, coff, w, xoff, subs)], NT) where subs =
    [(s0, Ns, yoff)]; xoff/yoff are packed-tensor column offsets.
    The shared group leads with small chunks so the critical first x
    prefetch is 0.5MB and the followers ramp up while compute starts;
    expert groups are one chunk each."""
    chunks = []
    off = xoff = yoff = 0
    for g in range(NG):
        plan = [256, 512, 768, 1024, 1024, 512] if g == 0 else [widths[g - 1]]
        for cw in plan:
            subs_w = _near_equal(cw, 512)
            if g == NG - 1 and cw == plan[-1] and subs_w[-1] > 128:
                # split a 64-col sliver off the globally-last sub: its
                # final y DMA is what the kernel drains on, so keep it tiny
                subs_w = subs_w[:-1] + [subs_w[-1] - 64, 64]
            subs = []
            s0 = 0
            for sw in subs_w:
                subs.append((s0, sw, yoff))
                s0 += sw
                yoff += DT * sw
            chunks.append((g, off, cw, xoff, subs))
            off += cw
            xoff += DT * cw
    return chunks, off


def _build_program(widths: tuple[int, ...]):
    """One SPMD program: 9 weight groups over [4096 | widths] columns."""
    chunks, NT = _chunk_list(widths)
    NCH = len(chunks)
    cwmax = max(c[2] for c in chunks)
    nc = bacc.Bacc()

    xt = nc.dram_tensor("xt", [P, DT * NT], _BF16, kind="ExternalInput")
    w1g = [nc.dram_tensor(f"w1_g{i}", [P, DT * FSL], _BF16, kind="ExternalInput")
           for i in range(NG)]
    w2g = [nc.dram_tensor(f"w2_g{i}", [P, FT * D], _BF16, kind="ExternalInput")
           for i in range(NG)]
    ballr = nc.dram_tensor("ball", [P, NG * FT], _F32, kind="ExternalInput")
    yt = nc.dram_tensor("yt", [P, DT * NT], _BF16, kind="ExternalOutput")

    with TileContext(nc) as tc:
        with (
            tc.tile_pool(name="wpool", bufs=3) as wpool,
            tc.tile_pool(name="xpool", bufs=2) as xpool,
            tc.tile_pool(name="hpool", bufs=8) as hpool,
            tc.tile_pool(name="ypool", bufs=3) as ypool,
            tc.tile_pool(name="bpool", bufs=1) as bpool,
            tc.tile_pool(name="psum", bufs=4, space="PSUM") as psum,
        ):
            # weight tiles rotate through 3 buffers per tag (groups are
            # consumed in order, so group g+2 can stream into group g-1's
            # slot while group g computes). w1 is packed f-major and w2
            # d-major, so the leading bytes of each are exactly what the
            # first accumulation groups consume — group 0's loads split
            # finer and straddle both DGE rings to minimize the startup
            # critical path.
            def load_w1(g):
                t = wpool.tile([P, DT * FSL], _BF16, tag="w1", name=f"w1_{g}")
                cuts = (0, DT * P, 2 * DT * P, DT * FSL) if g == 0 \
                    else (0, DT * FSL // 2, DT * FSL)
                for a, b2_ in zip(cuts[:-1], cuts[1:]):
                    nc.scalar.dma_start(t[:, a:b2_], w1g[g][:, a:b2_])
                return t

            def load_w2(g):
                t = wpool.tile([P, FT * D], _BF16, tag="w2", name=f"w2_{g}")
                if g == 0:
                    # per-d slices, straddling both rings so delivery keeps
                    # ahead of chunk 0's MM2 d-loop consumption
                    for d in range(DT):
                        eng = nc.sync if d < 5 else nc.scalar
                        eng.dma_start(t[:, d * FT * P:(d + 1) * FT * P],
                                      w2g[g][:, d * FT * P:(d + 1) * FT * P])
                else:
                    nc.scalar.dma_start(t, w2g[g][:, :])
                return t

            def load_x(ci):
                _, _, w, xoff, _ = chunks[ci]
                t = xpool.tile([P, DT * cwmax], _BF16, tag="x", name=f"x_{ci}")
                t = t[:, :DT * w]
                nc.sync.dma_start(t, xt[:, xoff:xoff + DT * w])
                return t

            # PE warm-up: dummy matmuls on a zeroed tile keep the PE busy
            # across the DMA delivery latency so the HAM clock-gate is at
            # 8/8 (2.4 GHz) when real matmuls issue.
            warm = bpool.tile([P, P + 512], _BF16, tag="warm", name="warm")
            nc.any.memset(warm[:, :], 0.0)
            wps = psum.tile([P, 512], _F32, tag="py", name="pwarm")
            for _ in range(N_WARM):
                nc.tensor.matmul(wps, lhsT=warm[:, :P], rhs=warm[:, P:],
                                 start=True, stop=True)

            # critical prefetch: x chunk 0 + ball + w2_g0 (SP ring) race
            # w1_g0 (ACT ring); the first matmul group needs only x0
            # (0.5MB) and w1_g0's leading f-tile quarter (0.25MB)
            x_next = load_x(0)
            ball = bpool.tile([P, NG * FT], _F32, tag="ballt", name="ballt")
            nc.sync.dma_start(ball, ballr[:, :])
            w1t: dict[int, object] = {}
            w2t: dict[int, object] = {}
            w1t[0] = load_w1(0)
            w2t[0] = load_w2(0)

            for ci, (g, coff, w, xoff, subs) in enumerate(chunks):
                xts = x_next
                if ci + 1 < NCH:
                    x_next = load_x(ci + 1)

                for si, (s0, Ns, yoff) in enumerate(subs):
                    hts = []
                    for f in range(FT):
                        ph = psum.tile([P, 512], _F32, tag="ph",
                                       name="ph")[:, :Ns]
                        for d in range(DT):
                            nc.tensor.matmul(
                                ph,
                                lhsT=w1t[g][:, (f * DT + d) * P:
                                            (f * DT + d + 1) * P],
                                rhs=xts[:, d * w + s0:d * w + s0 + Ns],
                                start=(d == 0),
                                stop=(d == DT - 1),
                            )
                        ht = hpool.tile([P, 512], _BF16, tag="h",
                                        name="h")[:, :Ns]
                        nc.scalar.activation(
                            ht, ph, mybir.ActivationFunctionType.Gelu,
                            bias=ball[:, g * FT + f:g * FT + f + 1],
                        )
                        hts.append(ht)

                    if si == 0:
                        # stream weights two groups ahead, issued AFTER the
                        # first gelus of this chunk: the ACT ring is FIFO,
                        # so this keeps the critical w1_g0/w2_g0 (and each
                        # chunk's gelus) ahead of the weight megabytes, and
                        # auto-throttles weight DMA to compute progress.
                        # Two-ahead + 3 rotating buffers means the slot
                        # being overwritten was freed a full group ago.
                        for g2 in (g + 1, g + 2):
                            if g2 < NG and g2 not in w1t:
                                w1t[g2] = load_w1(g2)
                                w2t[g2] = load_w2(g2)

                    yo = ypool.tile([P, DT * 512], _BF16, tag="y", name="y")
                    yo = yo[:, :DT * Ns]
                    for d in range(DT):
                        py = psum.tile([P, 512], _F32, tag="py",
                                       name="py")[:, :Ns]
                        for f in range(FT):
                            nc.tensor.matmul(
                                py,
                                lhsT=w2t[g][:, (d * FT + f) * P:
                                            (d * FT + f + 1) * P],
                                rhs=hts[f],
                                start=(f == 0),
                                stop=(f == FT - 1),
                            )
                        nc.vector.tensor_copy(yo[:, d * Ns:(d + 1) * Ns], py)
                    nc.sync.dma_start(yt[:, yoff:yoff + DT * Ns], yo)

    nc.finalize()
    return nc


def _get_program(widths: tuple[int, ...]):
    if widths not in _program_cache:
        _program_cache[widths] = _build_program(widths)
    return _program_cache[widths]


def _route(xf: np.ndarray, W_gate: np.ndarray):
    """Replicate the reference gate in float64 (selection margins are ~1e-5,
    far above fp32 rounding, so the top-2 sets match the fp32 reference)."""
    logits = xf.astype(np.float64) @ W_gate.astype(np.float64)
    m = logits.max(axis=-1, keepdims=True)
    p = np.exp(logits - m)
    p /= p.sum(axis=-1, keepdims=True)
    top_i = np.argsort(-p, axis=-1, kind="stable")[:, :TOPK]
    top_v = np.take_along_axis(p, top_i, axis=-1)
    top_v = top_v / top_v.sum(axis=-1, keepdims=True)
    return top_i, top_v.astype(np.float32)


def _pack_cols(xbf_cols: np.ndarray) -> np.ndarray:
    """[N tokens, D] -> packed [P, DT*N] (d-major, partition-contiguous)."""
    n = xbf_cols.shape[0]
    return xbf_cols.T.reshape(DT, P, n).transpose(1, 0, 2).reshape(P, DT * n)


def kernel(x, W_gate, W1, b1, W2, b2, Ws1, bs1, Ws2, bs2):
    x = np.asarray(x, np.float32)
    xf = x.reshape(T, D)
    top_i, top_v = _route(xf, np.asarray(W_gate, np.float32))

    # per-expert token lists
    idx = [np.nonzero((top_i == e).any(axis=1))[0] for e in range(E)]
    wgt = []
    for e in range(E):
        sel = top_i[idx[e]] == e  # [cnt, K] exactly one True per row
        wgt.append(top_v[idx[e]][sel].astype(np.float32))
    counts = np.array([len(i) for i in idx])
    we = [int(-(-c // 8) * 8) for c in counts]  # expert col widths, 8-aligned

    # group order: experts whose final sub-chunk is largest first, so the
    # globally-last sub (the exec-time tail) is the smallest one
    perm = sorted(range(E), key=lambda e: -_near_equal(we[e], 512)[-1])
    widths = tuple(we[e] for e in perm)
    chunks, NT = _chunk_list(widths)

    xbf = xf.astype(ml_dtypes.bfloat16)
    cols = np.zeros(NT, np.int64)
    cols[:T] = np.arange(T)
    goff = []
    off = 4096
    for j, e in enumerate(perm):
        goff.append(off)
        cols[off:off + counts[e]] = idx[e]
        off += we[e]
    xsel = xbf[cols]  # [NT, D]
    xtc = np.empty((P, DT * NT), ml_dtypes.bfloat16)
    for g, coff, w, xoff, subs in chunks:
        xtc[:, xoff:xoff + DT * w] = _pack_cols(xsel[coff:coff + w])

    W1 = np.asarray(W1, np.float32).astype(ml_dtypes.bfloat16)
    W2 = np.asarray(W2, np.float32).astype(ml_dtypes.bfloat16)
    Ws1b = np.asarray(Ws1, np.float32).astype(ml_dtypes.bfloat16)
    Ws2b = np.asarray(Ws2, np.float32).astype(ml_dtypes.bfloat16)
    b1f = np.asarray(b1, np.float32)
    bs1f = np.asarray(bs1, np.float32)

    def pack_w1(wslice):  # [D, FSL] -> [P, (f, d, c)] f-major
        return np.ascontiguousarray(
            wslice.reshape(DT, P, FT, P).transpose(1, 2, 0, 3)
            .reshape(P, DT * FSL))

    def pack_w2(wslice):  # [FSL, D] -> [P, (d, f, c)] d-major
        return np.ascontiguousarray(
            wslice.reshape(FT, P, DT, P).transpose(1, 2, 0, 3)
            .reshape(P, FT * D))

    in_maps = []
    for c in range(E):
        sl = slice(c * FSL, (c + 1) * FSL)
        m = {"xt": xtc}
        ball = np.zeros((P, NG * FT), np.float32)
        ball[:, :FT] = bs1f[sl].reshape(FT, P).T
        m["w1_g0"] = pack_w1(Ws1b[:, sl])
        m["w2_g0"] = pack_w2(Ws2b[sl, :])
        for j, e in enumerate(perm):
            m[f"w1_g{1 + j}"] = pack_w1(W1[e][:, sl])
            m[f"w2_g{1 + j}"] = pack_w2(W2[e][sl, :])
            ball[:, (1 + j) * FT:(2 + j) * FT] = b1f[e][sl].reshape(FT, P).T
        m["ball"] = ball
        in_maps.append(m)

    nc = _get_program(widths)
    global last_results
    last_results = run_bass_kernel_spmd(
        nc, in_maps, list(range(NCORES)), **TRACE_KWARGS)
    res = last_results.results

    ypacked = np.zeros((P, DT * NT), np.float32)
    for c in range(E):
        ypacked += np.asarray(res[c]["yt"], dtype=np.float32)
    # unpack per-sub [P, DT*Ns] blocks -> [NT, D]
    yfull = np.empty((NT, D), np.float32)
    for g, coff, w, xoff, subs in chunks:
        for s0, Ns, yoff in subs:
            blk = ypacked[:, yoff:yoff + DT * Ns].reshape(P, DT, Ns)
            yfull[coff + s0:coff + s0 + Ns] = blk.transpose(2, 1, 0).reshape(Ns, D)

    out = yfull[:T].copy()  # shared expert, all tokens
    for j, e in enumerate(perm):
        cnt = counts[e]
        out[idx[e]] += wgt[e][:, None] * yfull[goff[j]:goff[j] + cnt]

    # b2/bs2 enter linearly; add on host (zeros in this problem's inputs)
    b2 = np.asarray(b2, np.float32)
    bs2 = np.asarray(bs2, np.float32)
    combine = np.zeros((T, E), np.float32)
    np.put_along_axis(combine, top_i, top_v, axis=1)
    out += combine @ b2 + bs2

    return out.reshape(B, L, D)


# revision 34
# speedup vs baseline: 1.0055x; 1.0055x over previous
"""MoE block (8 experts, top-2, + shared expert) on 8 trn2 NeuronCores.

Strategy (FF-sharded tensor parallelism, host dispatch/combine):
  - Host computes gate logits/softmax/top-2 (0.03% of total FLOPs).
  - Every core receives the SAME activation matrix xt = [all 4096 tokens
    (shared-expert pass) | expert-0's routed tokens | ... | expert-7's],
    and a distinct 512-wide slice of the FF dim of EVERY weight matrix
    (8 experts + shared = 9 groups; slices stream through 3 rotating
    SBUF buffers since groups are consumed in order).
    gelu is elementwise over FF, so y = sum_c gelu(x@W1[:,c])@W2[c,:]
    decomposes exactly; the host sums the 8 partial outputs in fp32.
  - Per-core work is identical regardless of routing (no padding to the
    max expert count, no load imbalance): (12288+pad) cols x 1/8 of FF.
  - Matmuls are bf16 with fp32 PSUM accumulation; feature-major layout
    ([D, tokens]) avoids all on-device transposes. Outputs return bf16
    (partials are summed in fp32 on host; quantization adds ~1e-3 rel).

Granularity: x is loaded per CHUNK (up to a whole expert group, 13
loads total — each chunk-start matmul pays ~120ns for its x semaphore
wait, so fewer chunks = fewer stalls); compute and y stores run per
SUB-chunk (<=512 cols, the PSUM bank + matmul free-dim limit).

DMA layout: every dram tensor is PRE-PACKED on the host into the exact
SBUF tile byte order ([128 partitions, rest-contiguous]), so each
transfer is one contiguous ~8-17KB segment per partition (128
descriptors per transfer). The HWDGE generates descriptors inline on
the issuing sequencer's DIRECT2D at ~2.5-7ns each, so 1KB-segment
transfers (the naive [D, NT] layout) cap the whole kernel at ~150GB/s
of issue rate; packed transfers measured 345GB/s/core with all 8 cores
running. Weights stream on the ACT HWDGE ring, x/y on the SP ring,
keeping the periodic x prefetches out of FIFO order behind weight
megabytes.
"""

import numpy as np
import ml_dtypes

import concourse.bass as bass
import concourse.bacc as bacc
from concourse import mybir
from concourse.tile import TileContext
from concourse.bass_utils import run_bass_kernel_spmd

D = 1024
FF = 4096
E = 8
TOPK = 2
B, L = 4, 1024
T = B * L
NCORES = 8
P = 128
DT = D // P         # 8 k-tiles over D
FSL = FF // NCORES  # 512 FF columns per core
FT = FSL // P       # 4 f-tiles per core slice
NG = E + 1          # weight groups: 0 = shared, 1..8 = experts

_BF16 = mybir.dt.bfloat16
_F32 = mybir.dt.float32

_program_cache: dict[tuple, object] = {}

# test harness hooks: extra kwargs for run_bass_kernel_spmd (e.g. trace=True)
# and the last BassKernelResults for profiling. Unused in normal grading runs.
TRACE_KWARGS: dict = {}
last_results = None

N_WARM = 28  # dummy matmuls bridging the DMA delivery after kernel start
             # (sized for the slowest core's data arrival; exec time is
             # the max over cores, so the bridge covers arrival jitter)


def _near_equal(w: int, cap: int) -> list[int]:
    """Split w into ceil(w/cap) near-equal parts, multiples of 8."""
    n = -(-w // cap)
    base = -(-w // n)
    base = -(-base // 8) * 8
    sizes = []
    left = w
    for _ in range(n - 1):
        sizes.append(base)
        left -= base
    sizes.append(left)
    assert all(0 < s <= cap for s in sizes) and sum(sizes) == w
    return sizes


def _chunk_list(widths: tuple[int, ...]):
    """Chunks (x granularity) with sub-chunks (compute/y granularity).

    Returns ([(g, coff, w, xoff, subs)], NT) where subs =
    [(s0, Ns, yoff)]; xoff/yoff are packed-tensor column offsets.
    The shared group leads with small chunks so the critical first x
    prefetch is 0.5MB and the followers ramp up while compute starts;
    expert groups are one chunk each."""
    chunks = []
    off = xoff = yoff = 0
    for g in range(NG):
        plan = [512, 512, 1024, 1024, 1024] if g == 0 else [widths[g - 1]]
        for cw in plan:
            subs_w = _near_equal(cw, 512)
            if g == NG - 1 and cw == plan[-1] and subs_w[-1] > 128:
                # split a 64-col sliver off the globally-last sub: its
                # final y DMA is what the kernel drains on, so keep it tiny
                subs_w = subs_w[:-1] + [subs_w[-1] - 64, 64]
            subs = []
            s0 = 0
            for sw in subs_w:
                subs.append((s0, sw, yoff))
                s0 += sw
                yoff += DT * sw
            chunks.append((g, off, cw, xoff, subs))
            off += cw
            xoff += DT * cw
    return chunks, off


def _build_program(widths: tuple[int, ...]):
    """One SPMD program: 9 weight groups over [4096 | widths] columns."""
    chunks, NT = _chunk_list(widths)
    NCH = len(chunks)
    cwmax = max(c[2] for c in chunks)
    nc = bacc.Bacc()

    xt = nc.dram_tensor("xt", [P, DT * NT], _BF16, kind="ExternalInput")
    w1g = [nc.dram_tensor(f"w1_g{i}", [P, DT * FSL], _BF16, kind="ExternalInput")
           for i in range(NG)]
    w2g = [nc.dram_tensor(f"w2_g{i}", [P, FT * D], _BF16, kind="ExternalInput")
           for i in range(NG)]
    ballr = nc.dram_tensor("ball", [P, NG * FT], _F32, kind="ExternalInput")
    yt = nc.dram_tensor("yt", [P, DT * NT], _BF16, kind="ExternalOutput")

    with TileContext(nc) as tc:
        with (
            tc.tile_pool(name="wpool", bufs=3) as wpool,
            tc.tile_pool(name="xpool", bufs=2) as xpool,
            tc.tile_pool(name="hpool", bufs=8) as hpool,
            tc.tile_pool(name="ypool", bufs=3) as ypool,
            tc.tile_pool(name="bpool", bufs=1) as bpool,
            tc.tile_pool(name="psum", bufs=4, space="PSUM") as psum,
        ):
            # weight tiles rotate through 3 buffers per tag (groups are
            # consumed in order, so group g+2 can stream into group g-1's
            # slot while group g computes). w1 is packed f-major and w2
            # d-major, so the leading bytes of each are exactly what the
            # first accumulation groups consume — group 0's loads split
            # finer and straddle both DGE rings to minimize the startup
            # critical path.
            def load_w1(g):
                t = wpool.tile([P, DT * FSL], _BF16, tag="w1", name=f"w1_{g}")
                cuts = (0, DT * P, 2 * DT * P, DT * FSL) if g == 0 \
                    else (0, DT * FSL // 2, DT * FSL)
                for a, b2_ in zip(cuts[:-1], cuts[1:]):
                    nc.scalar.dma_start(t[:, a:b2_], w1g[g][:, a:b2_])
                return t

            def load_w2(g):
                t = wpool.tile([P, FT * D], _BF16, tag="w2", name=f"w2_{g}")
                if g == 0:
                    # per-d slices, straddling both rings so delivery keeps
                    # ahead of chunk 0's MM2 d-loop consumption
                    for d in range(DT):
                        eng = nc.sync if d < 5 else nc.scalar
                        eng.dma_start(t[:, d * FT * P:(d + 1) * FT * P],
                                      w2g[g][:, d * FT * P:(d + 1) * FT * P])
                else:
                    nc.scalar.dma_start(t, w2g[g][:, :])
                return t

            def load_x(ci):
                _, _, w, xoff, _ = chunks[ci]
                t = xpool.tile([P, DT * cwmax], _BF16, tag="x", name=f"x_{ci}")
                t = t[:, :DT * w]
                nc.sync.dma_start(t, xt[:, xoff:xoff + DT * w])
                return t

            # PE warm-up: dummy matmuls on a zeroed tile keep the PE busy
            # across the DMA delivery latency so the HAM clock-gate is at
            # 8/8 (2.4 GHz) when real matmuls issue.
            warm = bpool.tile([P, P + 512], _BF16, tag="warm", name="warm")
            nc.any.memset(warm[:, :], 0.0)
            wps = psum.tile([P, 512], _F32, tag="py", name="pwarm")
            for _ in range(N_WARM):
                nc.tensor.matmul(wps, lhsT=warm[:, :P], rhs=warm[:, P:],
                                 start=True, stop=True)

            # critical prefetch: x chunk 0 + ball + w2_g0 (SP ring) race
            # w1_g0 (ACT ring); the first matmul group needs only x0
            # (0.5MB) and w1_g0's leading f-tile quarter (0.25MB)
            x_next = load_x(0)
            ball = bpool.tile([P, NG * FT], _F32, tag="ballt", name="ballt")
            nc.sync.dma_start(ball, ballr[:, :])
            w1t: dict[int, object] = {}
            w2t: dict[int, object] = {}
            w1t[0] = load_w1(0)
            w2t[0] = load_w2(0)

            for ci, (g, coff, w, xoff, subs) in enumerate(chunks):
                xts = x_next
                if ci + 1 < NCH:
                    x_next = load_x(ci + 1)

                for si, (s0, Ns, yoff) in enumerate(subs):
                    hts = []
                    for f in range(FT):
                        ph = psum.tile([P, 512], _F32, tag="ph",
                                       name="ph")[:, :Ns]
                        for d in range(DT):
                            nc.tensor.matmul(
                                ph,
                                lhsT=w1t[g][:, (f * DT + d) * P:
                                            (f * DT + d + 1) * P],
                                rhs=xts[:, d * w + s0:d * w + s0 + Ns],
                                start=(d == 0),
                                stop=(d == DT - 1),
                            )
                        ht = hpool.tile([P, 512], _BF16, tag="h",
                                        name="h")[:, :Ns]
                        nc.scalar.activation(
                            ht, ph, mybir.ActivationFunctionType.Gelu,
                            bias=ball[:, g * FT + f:g * FT + f + 1],
                        )
                        hts.append(ht)

                    if si == 0:
                        # stream weights two groups ahead, issued AFTER the
                        # first gelus of this chunk: the ACT ring is FIFO,
                        # so this keeps the critical w1_g0/w2_g0 (and each
                        # chunk's gelus) ahead of the weight megabytes, and
                        # auto-throttles weight DMA to compute progress.
                        # Two-ahead + 3 rotating buffers means the slot
                        # being overwritten was freed a full group ago.
                        for g2 in (g + 1, g + 2):
                            if g2 < NG and g2 not in w1t:
                                w1t[g2] = load_w1(g2)
                                w2t[g2] = load_w2(g2)

                    yo = ypool.tile([P, DT * 512], _BF16, tag="y", name="y")
                    yo = yo[:, :DT * Ns]
                    for d in range(DT):
                        py = psum.tile([P, 512], _F32, tag="py",
                                       name="py")[:, :Ns]
                        for f in range(FT):
                            nc.tensor.matmul(
                                py,
                                lhsT=w2t[g][:, (d * FT + f) * P:
                                            (d * FT + f + 1) * P],
                                rhs=hts[f],
                                start=(f == 0),
                                stop=(f == FT - 1),
                            )
                        nc.vector.tensor_copy(yo[:, d * Ns:(d + 1) * Ns], py)
                    nc.sync.dma_start(yt[:, yoff:yoff + DT * Ns], yo)

    nc.finalize()
    return nc


def _get_program(widths: tuple[int, ...]):
    if widths not in _program_cache:
        _program_cache[widths] = _build_program(widths)
    return _program_cache[widths]


def _route(xf: np.ndarray, W_gate: np.ndarray):
    """Replicate the reference gate in float64 (selection margins are ~1e-5,
    far above fp32 rounding, so the top-2 sets match the fp32 reference)."""
    logits = xf.astype(np.float64) @ W_gate.astype(np.float64)
    m = logits.max(axis=-1, keepdims=True)
    p = np.exp(logits - m)
    p /= p.sum(axis=-1, keepdims=True)
    top_i = np.argsort(-p, axis=-1, kind="stable")[:, :TOPK]
    top_v = np.take_along_axis(p, top_i, axis=-1)
    top_v = top_v / top_v.sum(axis=-1, keepdims=True)
    return top_i, top_v.astype(np.float32)


def _pack_cols(xbf_cols: np.ndarray) -> np.ndarray:
    """[N tokens, D] -> packed [P, DT*N] (d-major, partition-contiguous)."""
    n = xbf_cols.shape[0]
    return xbf_cols.T.reshape(DT, P, n).transpose(1, 0, 2).reshape(P, DT * n)


def kernel(x, W_gate, W1, b1, W2, b2, Ws1, bs1, Ws2, bs2):
    x = np.asarray(x, np.float32)
    xf = x.reshape(T, D)
    top_i, top_v = _route(xf, np.asarray(W_gate, np.float32))

    # per-expert token lists
    idx = [np.nonzero((top_i == e).any(axis=1))[0] for e in range(E)]
    wgt = []
    for e in range(E):
        sel = top_i[idx[e]] == e  # [cnt, K] exactly one True per row
        wgt.append(top_v[idx[e]][sel].astype(np.float32))
    counts = np.array([len(i) for i in idx])
    we = [int(-(-c // 8) * 8) for c in counts]  # expert col widths, 8-aligned

    # group order: experts whose final sub-chunk is largest first, so the
    # globally-last sub (the exec-time tail) is the smallest one
    perm = sorted(range(E), key=lambda e: -_near_equal(we[e], 512)[-1])
    widths = tuple(we[e] for e in perm)
    chunks, NT = _chunk_list(widths)

    xbf = xf.astype(ml_dtypes.bfloat16)
    cols = np.zeros(NT, np.int64)
    cols[:T] = np.arange(T)
    goff = []
    off = 4096
    for j, e in enumerate(perm):
        goff.append(off)
        cols[off:off + counts[e]] = idx[e]
        off += we[e]
    xsel = xbf[cols]  # [NT, D]
    xtc = np.empty((P, DT * NT), ml_dtypes.bfloat16)
    for g, coff, w, xoff, subs in chunks:
        xtc[:, xoff:xoff + DT * w] = _pack_cols(xsel[coff:coff + w])

    W1 = np.asarray(W1, np.float32).astype(ml_dtypes.bfloat16)
    W2 = np.asarray(W2, np.float32).astype(ml_dtypes.bfloat16)
    Ws1b = np.asarray(Ws1, np.float32).astype(ml_dtypes.bfloat16)
    Ws2b = np.asarray(Ws2, np.float32).astype(ml_dtypes.bfloat16)
    b1f = np.asarray(b1, np.float32)
    bs1f = np.asarray(bs1, np.float32)

    def pack_w1(wslice):  # [D, FSL] -> [P, (f, d, c)] f-major
        return np.ascontiguousarray(
            wslice.reshape(DT, P, FT, P).transpose(1, 2, 0, 3)
            .reshape(P, DT * FSL))

    def pack_w2(wslice):  # [FSL, D] -> [P, (d, f, c)] d-major
        return np.ascontiguousarray(
            wslice.reshape(FT, P, DT, P).transpose(1, 2, 0, 3)
            .reshape(P, FT * D))

    in_maps = []
    for c in range(E):
        sl = slice(c * FSL, (c + 1) * FSL)
        m = {"xt": xtc}
        ball = np.zeros((P, NG * FT), np.float32)
        ball[:, :FT] = bs1f[sl].reshape(FT, P).T
        m["w1_g0"] = pack_w1(Ws1b[:, sl])
        m["w2_g0"] = pack_w2(Ws2b[sl, :])
        for j, e in enumerate(perm):
            m[f"w1_g{1 + j}"] = pack_w1(W1[e][:, sl])
            m[f"w2_g{1 + j}"] = pack_w2(W2[e][sl, :])
            ball[:, (1 + j) * FT:(2 + j) * FT] = b1f[e][sl].reshape(FT, P).T
        m["ball"] = ball
        in_maps.append(m)

    nc = _get_program(widths)
    global last_results
    last_results = run_bass_kernel_spmd(
        nc, in_maps, list(range(NCORES)), **TRACE_KWARGS)
    res = last_results.results

    ypacked = np.zeros((P, DT * NT), np.float32)
    for c in range(E):
        ypacked += np.asarray(res[c]["yt"], dtype=np.float32)
    # unpack per-sub [P, DT*Ns] blocks -> [NT, D]
    yfull = np.empty((NT, D), np.float32)
    for g, coff, w, xoff, subs in chunks:
        for s0, Ns, yoff in subs:
            blk = ypacked[:, yoff:yoff + DT * Ns].reshape(P, DT, Ns)
            yfull[coff + s0:coff + s0 + Ns] = blk.transpose(2, 1, 0).reshape(Ns, D)

    out = yfull[:T].copy()  # shared expert, all tokens
    for j, e in enumerate(perm):
        cnt = counts[e]
        out[idx[e]] += wgt[e][:, None] * yfull[goff[j]:goff[j] + cnt]

    # b2/bs2 enter linearly; add on host (zeros in this problem's inputs)
    b2 = np.asarray(b2, np.float32)
    bs2 = np.asarray(bs2, np.float32)
    combine = np.zeros((T, E), np.float32)
    np.put_along_axis(combine, top_i, top_v, axis=1)
    out += combine @ b2 + bs2

    return out.reshape(B, L, D)


# revision 35
# speedup vs baseline: 1.0128x; 1.0073x over previous
"""MoE block (8 experts, top-2, + shared expert) on 8 trn2 NeuronCores.

Strategy (FF-sharded tensor parallelism, host dispatch/combine):
  - Host computes gate logits/softmax/top-2 (0.03% of total FLOPs).
  - Every core receives the SAME activation matrix xt = [all 4096 tokens
    (shared-expert pass) | expert-0's routed tokens | ... | expert-7's],
    and a distinct 512-wide slice of the FF dim of EVERY weight matrix
    (8 experts + shared = 9 groups; slices stream through 3 rotating
    SBUF buffers since groups are consumed in order).
    gelu is elementwise over FF, so y = sum_c gelu(x@W1[:,c])@W2[c,:]
    decomposes exactly; the host sums the 8 partial outputs in fp32.
  - Per-core work is identical regardless of routing (no padding to the
    max expert count, no load imbalance): (12288+pad) cols x 1/8 of FF.
  - Matmuls are bf16 with fp32 PSUM accumulation; feature-major layout
    ([D, tokens]) avoids all on-device transposes. Outputs return bf16
    (partials are summed in fp32 on host; quantization adds ~1e-3 rel).

Granularity: x is loaded per CHUNK (up to a whole expert group, 13
loads total — each chunk-start matmul pays ~120ns for its x semaphore
wait, so fewer chunks = fewer stalls); compute and y stores run per
SUB-chunk (<=512 cols, the PSUM bank + matmul free-dim limit).

DMA layout: every dram tensor is PRE-PACKED on the host into the exact
SBUF tile byte order ([128 partitions, rest-contiguous]), so each
transfer is one contiguous ~8-17KB segment per partition (128
descriptors per transfer). The HWDGE generates descriptors inline on
the issuing sequencer's DIRECT2D at ~2.5-7ns each, so 1KB-segment
transfers (the naive [D, NT] layout) cap the whole kernel at ~150GB/s
of issue rate; packed transfers measured 345GB/s/core with all 8 cores
running. Weights stream on the ACT HWDGE ring, x/y on the SP ring,
keeping the periodic x prefetches out of FIFO order behind weight
megabytes.
"""

import numpy as np
import ml_dtypes

import concourse.bass as bass
import concourse.bacc as bacc
from concourse import mybir
from concourse.tile import TileContext
from concourse.bass_utils import run_bass_kernel_spmd

D = 1024
FF = 4096
E = 8
TOPK = 2
B, L = 4, 1024
T = B * L
NCORES = 8
P = 128
DT = D // P         # 8 k-tiles over D
FSL = FF // NCORES  # 512 FF columns per core
FT = FSL // P       # 4 f-tiles per core slice
NG = E + 1          # weight groups: 0 = shared, 1..8 = experts

_BF16 = mybir.dt.bfloat16
_F32 = mybir.dt.float32

_program_cache: dict[tuple, object] = {}

# test harness hooks: extra kwargs for run_bass_kernel_spmd (e.g. trace=True)
# and the last BassKernelResults for profiling. Unused in normal grading runs.
TRACE_KWARGS: dict = {}
last_results = None

N_WARM = 28  # dummy matmuls bridging the DMA delivery after kernel start
             # (sized for the slowest core's data arrival; exec time is
             # the max over cores, so the bridge covers arrival jitter)


def _near_equal(w: int, cap: int) -> list[int]:
    """Split w into ceil(w/cap) near-equal parts, multiples of 8."""
    n = -(-w // cap)
    base = -(-w // n)
    base = -(-base // 8) * 8
    sizes = []
    left = w
    for _ in range(n - 1):
        sizes.append(base)
        left -= base
    sizes.append(left)
    assert all(0 < s <= cap for s in sizes) and sum(sizes) == w
    return sizes


def _chunk_list(widths: tuple[int, ...]):
    """Chunks (x granularity) with sub-chunks (compute/y granularity).

    Returns ([(g, coff, w, xoff, subs)], NT) where subs =
    [(s0, Ns, yoff)]; xoff/yoff are packed-tensor column offsets.
    The shared group leads with small chunks so the critical first x
    prefetch is 0.5MB and the followers ramp up while compute starts;
    expert groups are one chunk each."""
    chunks = []
    off = xoff = yoff = 0
    for g in range(NG):
        plan = [512, 512, 1024, 1024, 1024] if g == 0 else [widths[g - 1]]
        for cw in plan:
            subs_w = _near_equal(cw, 512)
            if g == NG - 1 and cw == plan[-1] and subs_w[-1] > 128:
                # split a 64-col sliver off the globally-last sub: its
                # final y DMA is what the kernel drains on, so keep it tiny
                subs_w = subs_w[:-1] + [subs_w[-1] - 64, 64]
            subs = []
            s0 = 0
            for sw in subs_w:
                subs.append((s0, sw, yoff))
                s0 += sw
                yoff += DT * sw
            chunks.append((g, off, cw, xoff, subs))
            off += cw
            xoff += DT * cw
    return chunks, off


def _build_program(widths: tuple[int, ...]):
    """One SPMD program: 9 weight groups over [4096 | widths] columns."""
    chunks, NT = _chunk_list(widths)
    NCH = len(chunks)
    cwmax = max(c[2] for c in chunks)
    nc = bacc.Bacc()

    xt = nc.dram_tensor("xt", [P, DT * NT], _BF16, kind="ExternalInput")
    w1g = [nc.dram_tensor(f"w1_g{i}", [P, DT * FSL], _BF16, kind="ExternalInput")
           for i in range(NG)]
    w2g = [nc.dram_tensor(f"w2_g{i}", [P, FT * D], _BF16, kind="ExternalInput")
           for i in range(NG)]
    ballr = nc.dram_tensor("ball", [P, NG * FT], _F32, kind="ExternalInput")
    yt = nc.dram_tensor("yt", [P, DT * NT], _BF16, kind="ExternalOutput")

    with TileContext(nc) as tc:
        with (
            tc.tile_pool(name="wpool", bufs=3) as wpool,
            tc.tile_pool(name="xpool", bufs=2) as xpool,
            tc.tile_pool(name="hpool", bufs=8) as hpool,
            tc.tile_pool(name="ypool", bufs=3) as ypool,
            tc.tile_pool(name="bpool", bufs=1) as bpool,
            tc.tile_pool(name="psum", bufs=4, space="PSUM") as psum,
        ):
            # weight tiles rotate through 3 buffers per tag (groups are
            # consumed in order, so group g+2 can stream into group g-1's
            # slot while group g computes). w1 is packed f-major and w2
            # d-major, so the leading bytes of each are exactly what the
            # first accumulation groups consume — group 0's loads split
            # finer and straddle both DGE rings to minimize the startup
            # critical path.
            def load_w1(g):
                t = wpool.tile([P, DT * FSL], _BF16, tag="w1", name=f"w1_{g}")
                cuts = (0, DT * P, 2 * DT * P, DT * FSL) if g == 0 \
                    else (0, DT * FSL // 2, DT * FSL)
                for a, b2_ in zip(cuts[:-1], cuts[1:]):
                    nc.scalar.dma_start(t[:, a:b2_], w1g[g][:, a:b2_])
                return t

            def load_w2(g):
                t = wpool.tile([P, FT * D], _BF16, tag="w2", name=f"w2_{g}")
                if g == 0:
                    # per-d slices, straddling both rings so delivery keeps
                    # ahead of chunk 0's MM2 d-loop consumption
                    for d in range(DT):
                        eng = nc.sync if d < 5 else nc.scalar
                        eng.dma_start(t[:, d * FT * P:(d + 1) * FT * P],
                                      w2g[g][:, d * FT * P:(d + 1) * FT * P])
                else:
                    nc.scalar.dma_start(t, w2g[g][:, :])
                return t

            def load_x(ci):
                _, _, w, xoff, _ = chunks[ci]
                t = xpool.tile([P, DT * cwmax], _BF16, tag="x", name=f"x_{ci}")
                t = t[:, :DT * w]
                nc.sync.dma_start(t, xt[:, xoff:xoff + DT * w])
                return t

            # PE warm-up: dummy matmuls on a zeroed tile keep the PE busy
            # across the DMA delivery latency so the HAM clock-gate is at
            # 8/8 (2.4 GHz) when real matmuls issue.
            warm = bpool.tile([P, P + 512], _BF16, tag="warm", name="warm")
            nc.any.memset(warm[:, :], 0.0)
            wps = psum.tile([P, 512], _F32, tag="py", name="pwarm")
            for _ in range(N_WARM):
                nc.tensor.matmul(wps, lhsT=warm[:, :P], rhs=warm[:, P:],
                                 start=True, stop=True)

            # critical prefetch: x chunk 0 + ball + w2_g0 (SP ring) race
            # w1_g0 (ACT ring); the first matmul group needs only x0
            # (0.5MB) and w1_g0's leading f-tile quarter (0.25MB)
            x_next = load_x(0)
            ball = bpool.tile([P, NG * FT], _F32, tag="ballt", name="ballt")
            nc.sync.dma_start(ball, ballr[:, :])
            w1t: dict[int, object] = {}
            w2t: dict[int, object] = {}
            w1t[0] = load_w1(0)
            w2t[0] = load_w2(0)

            for ci, (g, coff, w, xoff, subs) in enumerate(chunks):
                xts = x_next
                if ci + 1 < NCH:
                    x_next = load_x(ci + 1)

                for si, (s0, Ns, yoff) in enumerate(subs):
                    hts = []
                    for f in range(FT):
                        ph = psum.tile([P, 512], _F32, tag="ph",
                                       name="ph")[:, :Ns]
                        for d in range(DT):
                            nc.tensor.matmul(
                                ph,
                                lhsT=w1t[g][:, (f * DT + d) * P:
                                            (f * DT + d + 1) * P],
                                rhs=xts[:, d * w + s0:d * w + s0 + Ns],
                                start=(d == 0),
                                stop=(d == DT - 1),
                            )
                        ht = hpool.tile([P, 512], _BF16, tag="h",
                                        name="h")[:, :Ns]
                        nc.scalar.activation(
                            ht, ph, mybir.ActivationFunctionType.Gelu,
                            bias=ball[:, g * FT + f:g * FT + f + 1],
                        )
                        hts.append(ht)

                    if si == 0:
                        # stream weights two groups ahead, issued AFTER the
                        # first gelus of this chunk: the ACT ring is FIFO,
                        # so this keeps the critical w1_g0/w2_g0 (and each
                        # chunk's gelus) ahead of the weight megabytes, and
                        # auto-throttles weight DMA to compute progress.
                        # Two-ahead + 3 rotating buffers means the slot
                        # being overwritten was freed a full group ago.
                        for g2 in (g + 1, g + 2):
                            if g2 < NG and g2 not in w1t:
                                w1t[g2] = load_w1(g2)
                                w2t[g2] = load_w2(g2)

                    yo = ypool.tile([P, DT * 512], _BF16, tag="y", name="y")
                    yo = yo[:, :DT * Ns]
                    for d in range(DT):
                        py = psum.tile([P, 512], _F32, tag="py",
                                       name="py")[:, :Ns]
                        for f in range(FT):
                            nc.tensor.matmul(
                                py,
                                lhsT=w2t[g][:, (d * FT + f) * P:
                                            (d * FT + f + 1) * P],
                                rhs=hts[f],
                                start=(f == 0),
                                stop=(f == FT - 1),
                            )
                        nc.vector.tensor_copy(yo[:, d * Ns:(d + 1) * Ns], py)
                    nc.sync.dma_start(yt[:, yoff:yoff + DT * Ns], yo)

    nc.finalize()
    return nc


def _get_program(widths: tuple[int, ...]):
    if widths not in _program_cache:
        _program_cache[widths] = _build_program(widths)
    return _program_cache[widths]


def _route(xf: np.ndarray, W_gate: np.ndarray):
    """Replicate the reference gate in float64 (selection margins are ~1e-5,
    far above fp32 rounding, so the top-2 sets match the fp32 reference)."""
    logits = xf.astype(np.float64) @ W_gate.astype(np.float64)
    m = logits.max(axis=-1, keepdims=True)
    p = np.exp(logits - m)
    p /= p.sum(axis=-1, keepdims=True)
    top_i = np.argsort(-p, axis=-1, kind="stable")[:, :TOPK]
    top_v = np.take_along_axis(p, top_i, axis=-1)
    top_v = top_v / top_v.sum(axis=-1, keepdims=True)
    return top_i, top_v.astype(np.float32)


def _pack_cols(xbf_cols: np.ndarray) -> np.ndarray:
    """[N tokens, D] -> packed [P, DT*N] (d-major, partition-contiguous)."""
    n = xbf_cols.shape[0]
    return xbf_cols.T.reshape(DT, P, n).transpose(1, 0, 2).reshape(P, DT * n)


def kernel(x, W_gate, W1, b1, W2, b2, Ws1, bs1, Ws2, bs2):
    x = np.asarray(x, np.float32)
    xf = x.reshape(T, D)
    top_i, top_v = _route(xf, np.asarray(W_gate, np.float32))

    # per-expert token lists
    idx = [np.nonzero((top_i == e).any(axis=1))[0] for e in range(E)]
    wgt = []
    for e in range(E):
        sel = top_i[idx[e]] == e  # [cnt, K] exactly one True per row
        wgt.append(top_v[idx[e]][sel].astype(np.float32))
    counts = np.array([len(i) for i in idx])
    # expert col widths, 8-aligned, min 8 (keeps the chunk plan well-formed
    # even if some expert receives no tokens)
    we = [max(8, int(-(-c // 8) * 8)) for c in counts]

    # group order: experts whose final sub-chunk is largest first, so the
    # globally-last sub (the exec-time tail) is the smallest one
    perm = sorted(range(E), key=lambda e: -_near_equal(we[e], 512)[-1])
    widths = tuple(we[e] for e in perm)
    chunks, NT = _chunk_list(widths)

    xbf = xf.astype(ml_dtypes.bfloat16)
    cols = np.zeros(NT, np.int64)
    cols[:T] = np.arange(T)
    goff = []
    off = 4096
    for j, e in enumerate(perm):
        goff.append(off)
        cols[off:off + counts[e]] = idx[e]
        off += we[e]
    xsel = xbf[cols]  # [NT, D]
    xtc = np.empty((P, DT * NT), ml_dtypes.bfloat16)
    for g, coff, w, xoff, subs in chunks:
        xtc[:, xoff:xoff + DT * w] = _pack_cols(xsel[coff:coff + w])

    W1 = np.asarray(W1, np.float32).astype(ml_dtypes.bfloat16)
    W2 = np.asarray(W2, np.float32).astype(ml_dtypes.bfloat16)
    Ws1b = np.asarray(Ws1, np.float32).astype(ml_dtypes.bfloat16)
    Ws2b = np.asarray(Ws2, np.float32).astype(ml_dtypes.bfloat16)
    b1f = np.asarray(b1, np.float32)
    bs1f = np.asarray(bs1, np.float32)

    def pack_w1(wslice):  # [D, FSL] -> [P, (f, d, c)] f-major
        return np.ascontiguousarray(
            wslice.reshape(DT, P, FT, P).transpose(1, 2, 0, 3)
            .reshape(P, DT * FSL))

    def pack_w2(wslice):  # [FSL, D] -> [P, (d, f, c)] d-major
        return np.ascontiguousarray(
            wslice.reshape(FT, P, DT, P).transpose(1, 2, 0, 3)
            .reshape(P, FT * D))

    in_maps = []
    for c in range(E):
        sl = slice(c * FSL, (c + 1) * FSL)
        m = {"xt": xtc}
        ball = np.zeros((P, NG * FT), np.float32)
        ball[:, :FT] = bs1f[sl].reshape(FT, P).T
        m["w1_g0"] = pack_w1(Ws1b[:, sl])
        m["w2_g0"] = pack_w2(Ws2b[sl, :])
        for j, e in enumerate(perm):
            m[f"w1_g{1 + j}"] = pack_w1(W1[e][:, sl])
            m[f"w2_g{1 + j}"] = pack_w2(W2[e][sl, :])
            ball[:, (1 + j) * FT:(2 + j) * FT] = b1f[e][sl].reshape(FT, P).T
        m["ball"] = ball
        in_maps.append(m)

    nc = _get_program(widths)
    global last_results
    last_results = run_bass_kernel_spmd(
        nc, in_maps, list(range(NCORES)), **TRACE_KWARGS)
    res = last_results.results

    ypacked = np.zeros((P, DT * NT), np.float32)
    for c in range(E):
        ypacked += np.asarray(res[c]["yt"], dtype=np.float32)
    # unpack per-sub [P, DT*Ns] blocks -> [NT, D]
    yfull = np.empty((NT, D), np.float32)
    for g, coff, w, xoff, subs in chunks:
        for s0, Ns, yoff in subs:
            blk = ypacked[:, yoff:yoff + DT * Ns].reshape(P, DT, Ns)
            yfull[coff + s0:coff + s0 + Ns] = blk.transpose(2, 1, 0).reshape(Ns, D)

    out = yfull[:T].copy()  # shared expert, all tokens
    for j, e in enumerate(perm):
        cnt = counts[e]
        out[idx[e]] += wgt[e][:, None] * yfull[goff[j]:goff[j] + cnt]

    # b2/bs2 enter linearly; add on host (zeros in this problem's inputs)
    b2 = np.asarray(b2, np.float32)
    bs2 = np.asarray(bs2, np.float32)
    combine = np.zeros((T, E), np.float32)
    np.put_along_axis(combine, top_i, top_v, axis=1)
    out += combine @ b2 + bs2

    return out.reshape(B, L, D)
